# revision 4
# baseline (speedup 1.0000x reference)
"""DLRM DotInteractionArch kernel for 8x Trainium2 NeuronCores.

Problem: B=16384, 26 sparse embeddings + 1 dense feature, D=128.
  combined[b] = concat(dense[b], emb[b])           # [27, 128]
  G[b] = combined[b] @ combined[b].T               # [27, 27]
  out[b] = concat(dense[b], triu(G[b], k=1).flat)  # [479]

V2 strategy (pure data parallel, 2048 samples/core, 16 rounds x 128 samples):
  - Loads are SAMPLE-MAJOR: one SWDGE cast-DMA per round per tensor
    ([128 b-partitions, 26*128] fp32->bf16, 128 fat descriptors) instead of
    f-major stride-4-partition loads (3456 descriptors + 828ns Q7 emission
    per DMA in the old layout).
  - PE transpose pass: 27 is_transpose matmuls [128b,128d] -> PSUM *bf16*
    (transpose keeps input dtype), so the PSUM->SBUF evacuation runs at the
    DVE 16-bit rate instead of fp32 CAST rate. Produces CT [128 d, 27 f, 128 b].
  - G-pass identical to V1: per-sample col-tiled matmuls (tile_position
    (0,32s)) -> PSG [32s+f, q, g'] fp32, 16ns/MM measured.
  - G evacuation on the Scalar engine (ACT) which is otherwise idle.
  - Stores are SLAB-BATCHED: triangle runs cover 7/7/2-round slabs so each
    run-f DMA carries 896 descriptors and the ~460ns fixed SWDGE emission
    cost is amortized 7x. Slab-0/1 stores are spread over later rounds'
    program order; the small slab-2 plus slab-1 remainder form the tail,
    split across gpsimd/sync/scalar issuers.
  - Dense passthrough: one HBM->HBM DMA on the sync HWDGE ring.

NOTE: stride-partition APs (ST[f::32]) are invisible to the Tile shadow-memory
dependency tracker, so store RAW edges on the slab staging are wired
explicitly with add_dep_helper. Slab staging tiles are never reused (three
disjoint tiles), so no WAR edges are needed.

Sample mapping per core: b = r*128 + g*4 + s  (r: round, g = h*16+q, s: 0..3)
"""

import numpy as np

B_FULL = 16384
N_CORES = 8
BC = B_FULL // N_CORES  # 2048 samples per core
F = 27                  # 1 dense + 26 sparse features
D = 128
NSPARSE = 26
SPR = 128               # samples per round
PAIRS = F * (F - 1) // 2  # 351
OUTC = D + PAIRS          # 479

_CACHE = {}


def _triu_offsets():
    # off[f] = column in the output row where G[f, f+1:] lands
    off = [D]
    for f in range(F - 1):
        off.append(off[-1] + (NSPARSE - f))
    return off


def _build_nc(bc: int = BC):
    from contextlib import ExitStack

    import concourse.bacc as bacc
    import concourse.tile as tile
    from concourse import mybir
    from concourse.masks import make_identity
    from concourse.tile_rust import add_dep_helper

    BF = mybir.dt.bfloat16
    F32 = mybir.dt.float32
    R = bc // SPR  # 16 rounds

    # store slabs in rounds: [0,7) [7,14) [14,16)
    SLAB_BOUNDS = [(0, 7), (7, 14), (14, 16)]

    nc = bacc.Bacc("TRN2", target_bir_lowering=False, debug=False)
    den = nc.dram_tensor("dense_output", [bc, D], F32, kind="ExternalInput")
    emb = nc.dram_tensor("embeddings", [bc, NSPARSE, D], F32, kind="ExternalInput")
    out = nc.dram_tensor("out", [bc, OUTC], F32, kind="ExternalOutput")

    off = _triu_offsets()

    emb_v = emb.ap().rearrange("(r p) j d -> r p j d", p=SPR)
    den_v = den.ap().rearrange("(r p) d -> r p d", p=SPR)

    with tile.TileContext(nc) as tc, ExitStack() as ctx:
        const = ctx.enter_context(tc.tile_pool(name="const", bufs=1))
        xep = ctx.enter_context(tc.tile_pool(name="xe", bufs=3))
        xdp = ctx.enter_context(tc.tile_pool(name="xd", bufs=3))
        ctp = ctx.enter_context(tc.tile_pool(name="ct", bufs=3))
        stp = ctx.enter_context(tc.tile_pool(name="st", bufs=1))
        ptp = ctx.enter_context(tc.tile_pool(name="pt", bufs=3, space="PSUM"))
        psgp = ctx.enter_context(tc.tile_pool(name="psg", bufs=4, space="PSUM"))

        ident = const.tile([128, 128], BF)
        make_identity(nc, ident)

        # dense passthrough columns: one HBM->HBM DMA on the sync HWDGE ring
        nc.sync.dma_start(out=out.ap()[:, 0:D], in_=den.ap()[:, :])

        # slab staging tiles (disjoint; never reused -> no WAR hazards)
        st_tiles = [
            stp.tile([128, r1 - r0, 2, 16, 32], F32, name=f"st{i}", tag=f"st{i}")
            for i, (r0, r1) in enumerate(SLAB_BOUNDS)
        ]
        st_copies = {i: [] for i in range(len(SLAB_BOUNDS))}  # slab -> evac insts

        def slab_of(r):
            for i, (r0, r1) in enumerate(SLAB_BOUNDS):
                if r0 <= r < r1:
                    return i, r - r0
            raise AssertionError

        def emit_store(slab, f, eng):
            r0, r1 = SLAB_BOUNDS[slab]
            n = NSPARSE - f
            src = st_tiles[slab][f::32, :, :, :, f + 1:F]
            dst = (
                out.ap()[r0 * SPR:r1 * SPR, off[f]:off[f] + n]
                .rearrange("(r h q s) c -> s r h q c", h=2, q=16, s=4)
            )
            d = eng.dma_start(out=dst, in_=src)
            for cpy in st_copies[slab]:
                add_dep_helper(d.ins, cpy.ins, reason="triu DMA RAW on ST")
            return d

        # store emission schedule: round -> list of (slab, f)
        sched = {}
        # slab 0 (ready after round 6) spread over rounds 7..12: 5,5,4,4,4,4
        counts = [5, 5, 4, 4, 4, 4]
        fi = 0
        for i, c in enumerate(counts):
            for _ in range(c):
                sched.setdefault(7 + i, []).append((0, fi))
                fi += 1
        assert fi == NSPARSE
        # slab 1 (ready after round 13): 5 at r14, 5 at r15, 16 in the tail
        fi = 0
        for i, c in enumerate([5, 5]):
            for _ in range(c):
                sched.setdefault(14 + i, []).append((1, fi))
                fi += 1
        slab1_tail = list(range(fi, NSPARSE))

        for r in range(R):
            slab, rs = slab_of(r)

            # ---- stores scheduled for this round (data from finished slabs)
            for (sl, f) in sched.get(r, []):
                emit_store(sl, f, nc.gpsimd)

            # ---- loads: sample-major, cast fp32->bf16 on SWDGE
            XE = xep.tile([128, NSPARSE, D], BF)
            XD = xdp.tile([128, D], BF)
            nc.gpsimd.dma_start(out=XE[:], in_=emb_v[r])
            nc.gpsimd.dma_start(out=XD[:], in_=den_v[r])

            # ---- T-pass: 27 transposes [128 b, 128 d] -> PT bf16 [128 d, 128 b]
            CT = ctp.tile([128, F, 128], BF)
            for t in range(7):
                js = list(range(4 * t, min(4 * t + 4, F)))
                PT = ptp.tile([128, 4, 128], BF)
                for i, j in enumerate(js):
                    src = XD[:] if j == 0 else XE[:, j - 1]
                    nc.tensor.transpose(PT[:, i], src, ident[:])
                nc.vector.tensor_copy(CT[:, js[0]:js[-1] + 1], PT[:, 0:len(js)])

            # ---- G-pass: per-sample col-tiled matmuls -> PSG [32s+f, q, g']
            for h in range(2):
                PSG = psgp.tile([128, 16, 32], F32)
                for q in range(16):
                    g = h * 16 + q
                    for s in range(4):
                        c = CT[:, :, 4 * g + s]  # [128 d, 27 f]
                        nc.tensor.matmul(
                            PSG[32 * s:32 * s + F, q, 0:F],
                            c,
                            c,
                            start=True,
                            stop=True,
                            tile_position=(0, 32 * s),
                        )
                cpy = nc.scalar.copy(st_tiles[slab][:, rs, h], PSG[:])
                st_copies[slab].append(cpy)

        # ---- tail: slab-1 remainder on gpsimd; slab-2 split 3 ways
        for f in slab1_tail:
            emit_store(1, f, nc.gpsimd)
        engs = [nc.gpsimd, nc.sync, nc.scalar]
        for f in range(NSPARSE):
            emit_store(2, f, engs[f % 3])

    nc.finalize()
    return nc


def kernel(dense_output: np.ndarray, embeddings: np.ndarray) -> np.ndarray:
    from concourse.bass_utils import run_bass_kernel_spmd

    if "nc" not in _CACHE:
        _CACHE["nc"] = _build_nc()
    nc = _CACHE["nc"]

    dense_output = np.ascontiguousarray(np.asarray(dense_output, dtype=np.float32))
    embeddings = np.ascontiguousarray(np.asarray(embeddings, dtype=np.float32))
    in_maps = []
    for i in range(N_CORES):
        sl = slice(i * BC, (i + 1) * BC)
        in_maps.append(
            {
                "dense_output": np.ascontiguousarray(dense_output[sl]),
                "embeddings": np.ascontiguousarray(embeddings[sl]),
            }
        )
    res = run_bass_kernel_spmd(nc, in_maps, list(range(N_CORES)))
    return np.concatenate([res.results[i]["out"] for i in range(N_CORES)], axis=0)


# revision 7
# speedup vs baseline: 1.3580x; 1.3580x over previous
"""DLRM DotInteractionArch kernel for 8x Trainium2 NeuronCores.

Problem: B=16384, 26 sparse embeddings + 1 dense feature, D=128.
  combined[b] = concat(dense[b], emb[b])           # [27, 128]
  G[b] = combined[b] @ combined[b].T               # [27, 27]
  out[b] = concat(dense[b], triu(G[b], k=1).flat)  # [479]

V3 strategy (pure data parallel, 2048 samples/core, 16 rounds x 128 samples):
  - Loads are SAMPLE-MAJOR: one SWDGE cast-DMA per round per tensor
    ([128 b-partitions, 26*128] fp32->bf16, 128 fat descriptors ~13KB each)
    -> Q7 emission ~0.6us/DMA instead of 110us total in the f-major layout.
  - PE transpose pass: 27 is_transpose matmuls [128b,128d] -> PSUM bf16
    (transpose keeps input dtype). DVE evacuates to CT [128 d, 128 b, 32
    f-slot] bf16 -- per-sample f-contiguous so the G matmul operands are
    contiguous (fast weight load + streaming).
  - G-pass: per-sample col-tiled matmuls (tile_position (0,32s)) ->
    PSG [32s+f, q, g'] fp32. Evacuated to per-round ST staging by the
    Scalar engine (ACT), which is otherwise idle.
  - Stores: per-round 26 triangle-run DMAs (many small concurrent transfers
    pipeline well at the SDMA level), split across the three issuers
    (sync/scalar HWDGE + gpsimd SWDGE). The gpsimd share is deferred one
    round so its emission never blocks the next round's loads in the Q7
    FIFO. Dense passthrough is one HBM->HBM DMA on the sync ring.

NOTE: stride-partition APs (ST[f::32]) are invisible to the Tile shadow-memory
dependency tracker, so RAW/WAR edges around the store DMAs are wired
explicitly with add_dep_helper.

Sample mapping per core: b = r*128 + g*4 + s  (r: round, g = h*16+q, s: 0..3)
"""

import numpy as np

B_FULL = 16384
N_CORES = 8
BC = B_FULL // N_CORES  # 2048 samples per core
F = 27                  # 1 dense + 26 sparse features
D = 128
NSPARSE = 26
SPR = 128               # samples per round
PAIRS = F * (F - 1) // 2  # 351
OUTC = D + PAIRS          # 479

_CACHE = {}


def _triu_offsets():
    off = [D]
    for f in range(F - 1):
        off.append(off[-1] + (NSPARSE - f))
    return off


def _build_nc(bc: int = BC):
    from contextlib import ExitStack

    import concourse.bacc as bacc
    import concourse.tile as tile
    from concourse import mybir
    from concourse.masks import make_identity
    from concourse.tile_rust import add_dep_helper

    BF = mybir.dt.bfloat16
    F32 = mybir.dt.float32
    R = bc // SPR  # 16 rounds

    nc = bacc.Bacc("TRN2", target_bir_lowering=False, debug=False)
    den = nc.dram_tensor("dense_output", [bc, D], F32, kind="ExternalInput")
    emb = nc.dram_tensor("embeddings", [bc, NSPARSE, D], F32, kind="ExternalInput")
    out = nc.dram_tensor("out", [bc, OUTC], F32, kind="ExternalOutput")

    off = _triu_offsets()

    emb_v = emb.ap().rearrange("(r p) j d -> r p j d", p=SPR)
    den_v = den.ap().rearrange("(r p) d -> r p d", p=SPR)
    out_v = out.ap().rearrange("(r h q s) c -> r s h q c", h=2, q=16, s=4)

    # issuer split for the 26 triangle-run stores: f -> engine kind
    # gpsimd share deferred one round (Q7 FIFO); sync/scalar immediate.
    GPS_RUNS = set(range(18, 26))       # 8 short runs on SWDGE
    ST_BUFS = 4

    with tile.TileContext(nc) as tc, ExitStack() as ctx:
        const = ctx.enter_context(tc.tile_pool(name="const", bufs=1))
        xep = ctx.enter_context(tc.tile_pool(name="xe", bufs=3))
        xdp = ctx.enter_context(tc.tile_pool(name="xd", bufs=3))
        ctp = ctx.enter_context(tc.tile_pool(name="ct", bufs=3))
        stp = ctx.enter_context(tc.tile_pool(name="st", bufs=ST_BUFS))
        ptp = ctx.enter_context(tc.tile_pool(name="pt", bufs=3, space="PSUM"))
        psgp = ctx.enter_context(tc.tile_pool(name="psg", bufs=4, space="PSUM"))

        ident = const.tile([128, 128], BF)
        make_identity(nc, ident)

        # dense passthrough columns: one HBM->HBM DMA
        nc.sync.dma_start(out=out.ap()[:, 0:D], in_=den.ap()[:, :])

        st_copies = {}       # round -> [evac insts]
        st_dmas = {}         # round -> [store insts]
        deferred_stores = []  # (round, f) for the gpsimd share

        def emit_store(r, f, eng):
            n = NSPARSE - f
            src = ST_tiles[r][f::32, :, :, f + 1:F]
            dst = out_v[r][:, :, :, off[f]:off[f] + n]
            d = eng.dma_start(out=dst, in_=src)
            st_dmas.setdefault(r, []).append(d)
            for cpy in st_copies[r]:
                add_dep_helper(d.ins, cpy.ins, reason="triu DMA RAW on ST")
            return d

        ST_tiles = {}
        for r in range(R):
            # ---- flush deferred gpsimd stores (previous round's data) ----
            for (rr, f) in deferred_stores:
                emit_store(rr, f, nc.gpsimd)
            deferred_stores = []

            # ---- loads: sample-major, cast fp32->bf16 on SWDGE ----
            XE = xep.tile([128, NSPARSE, D], BF)
            XD = xdp.tile([128, D], BF)
            nc.gpsimd.dma_start(out=XE[:], in_=emb_v[r])
            nc.gpsimd.dma_start(out=XD[:], in_=den_v[r])

            # ---- T-pass: 27 transposes [128 b, 128 d] -> PT bf16 [128 d, 128 b]
            CT = ctp.tile([128, F, 128], BF)  # [d, f, b]
            for t in range(7):
                js = list(range(4 * t, min(4 * t + 4, F)))
                PT = ptp.tile([128, 4, 128], BF)
                for i, j in enumerate(js):
                    src = XD[:] if j == 0 else XE[:, j - 1]
                    nc.tensor.transpose(PT[:, i], src, ident[:])
                nc.vector.tensor_copy(CT[:, js[0]:js[-1] + 1], PT[:, 0:len(js)])

            # ---- G-pass: per-sample col-tiled matmuls -> PSG [32s+f, q, g']
            ST = stp.tile([128, 2, 16, 32], F32)
            ST_tiles[r] = ST
            st_copies[r] = []
            for h in range(2):
                PSG = psgp.tile([128, 16, 32], F32)
                for q in range(16):
                    g = h * 16 + q
                    for s in range(4):
                        c = CT[:, :, 4 * g + s]  # [128 d, 27 f]
                        nc.tensor.matmul(
                            PSG[32 * s:32 * s + F, q, 0:F],
                            c,
                            c,
                            start=True,
                            stop=True,
                            tile_position=(0, 32 * s),
                        )
                cpy = nc.scalar.copy(ST[:, h], PSG[:])
                st_copies[r].append(cpy)
                # WAR: this evac reuses the ST slot read by round r-ST_BUFS DMAs
                for d in st_dmas.get(r - ST_BUFS, []):
                    add_dep_helper(cpy.ins, d.ins, reason="ST slot WAR")

            # ---- stores for this round: sync/scalar immediate, gpsimd deferred
            for f in range(F - 1):
                if f in GPS_RUNS:
                    deferred_stores.append((r, f))
                else:
                    eng = nc.sync if f % 2 == 0 else nc.scalar
                    emit_store(r, f, eng)

        # flush the last round's deferred stores
        for (rr, f) in deferred_stores:
            emit_store(rr, f, nc.gpsimd)

    nc.finalize()
    return nc


def kernel(dense_output: np.ndarray, embeddings: np.ndarray) -> np.ndarray:
    from concourse.bass_utils import run_bass_kernel_spmd

    if "nc" not in _CACHE:
        _CACHE["nc"] = _build_nc()
    nc = _CACHE["nc"]

    dense_output = np.ascontiguousarray(np.asarray(dense_output, dtype=np.float32))
    embeddings = np.ascontiguousarray(np.asarray(embeddings, dtype=np.float32))
    in_maps = []
    for i in range(N_CORES):
        sl = slice(i * BC, (i + 1) * BC)
        in_maps.append(
            {
                "dense_output": np.ascontiguousarray(dense_output[sl]),
                "embeddings": np.ascontiguousarray(embeddings[sl]),
            }
        )
    res = run_bass_kernel_spmd(nc, in_maps, list(range(N_CORES)))
    return np.concatenate([res.results[i]["out"] for i in range(N_CORES)], axis=0)
